# revision 9
# baseline (speedup 1.0000x reference)
"""DoRA multihead attention TRN2 kernel, v3: row-tiled S head pairs.

Per core (4 heads = 2 pairs, one batch). Heads of a pair live in partition
halves (rows 0-63 / 64-127) of shared q/k tiles, so the two K=64 S matmuls
run CONCURRENTLY in disjoint PE row-groups (tile_position via base_partition)
-- S costs half the streaming cycles of the padded-K=128 form.

Schedule (PSUM banks: S rotation 4 + AV half-sets 4 = 8):
  lead:   DMA loads, q-proj pair0 (n0..3), k-proj pair0 (n0)
  A(j):   S(P0,j) 2 halves; v-proj (2/round, rounds 0-7); AV(h0,j-2).half0;
          AV(h1,j-2).half0; proj filler (k P0 n1-3, q/k P1)
  B(j):   AV(h0,j).half1; AV(h1,j).half1; S(P1,j)
  C:      AV(h2/h3) both halves (8 banks), out-projection, DMA out
ACT does exp only (the global bottleneck: ~133us); all other PSUM
evacuation is on DVE/gpsimd.
"""
import sys
if "/opt/trn_rl_repo" not in sys.path:
    sys.path.insert(0, "/opt/trn_rl_repo")

import numpy as np
import ml_dtypes
from contextlib import ExitStack

import concourse.bass as bass
import concourse.tile as tile
from concourse import bacc, mybir

bf = ml_dtypes.bfloat16
BF16, F32 = mybir.dt.bfloat16, mybir.dt.float32
ActFn = mybir.ActivationFunctionType

T = 2048
E = 1024
HD = 64
NHL = 4                 # heads per core
NP = 2                  # head pairs per core
HSL = NHL * HD          # 256
VW = NHL * (HD + 1)     # 260
NT = T // 128           # 16
NF = E // 128           # 8
NI = T // 512           # 4


def build_body(tc, qT, wq, wk, wv, wo, bq, bk, bva, y):
    nc = tc.nc
    with ExitStack() as ctx:
        const = ctx.enter_context(tc.tile_pool(name="const", bufs=1))
        pss = ctx.enter_context(tc.tile_pool(name="pss", bufs=2, space="PSUM"))
        psav = ctx.enter_context(tc.tile_pool(name="psav", bufs=4, space="PSUM"))
        ppool = ctx.enter_context(tc.tile_pool(name="pT", bufs=64))
        # weights/input pool closed after round 8 of loop A; everything only
        # touched later (oT, wo, y tiles, recip/broadcast pools) lives in
        # pools opened after the close, reusing this 36KB/partition.
        wpool_cm = tc.tile_pool(name="wts", bufs=1)
        wpool = wpool_cm.__enter__()

        # ---- input loads ----
        bva_bc = const.tile([128, VW], BF16, tag="bva_bc")
        wq_all = const.tile([128, NF * HSL], BF16, tag="wq", name="wq_all")
        nc.sync.dma_start(
            wq_all[:], wq.rearrange("(f p) c -> p f c", p=128))
        bq_s = const.tile([128, 2], F32, tag="bq")
        nc.sync.dma_start(bq_s[:], bq[:])
        qk_all = wpool.tile([128, NF * T], BF16, tag="qTs", name="qk_all")
        qk = [qk_all[:, f * T:(f + 1) * T] for f in range(NF)]

        def qk_wave(n):
            nc.sync.dma_start(
                qk_all[:].rearrange("p (f c) -> p f c", f=NF)[:, :, n * 512:(n + 1) * 512],
                qT[:, n * 512:(n + 1) * 512].rearrange("(f p) c -> p f c", p=128))

        # wave 0 split into f-halves so the first projection group's f=0..3
        # matmuls start as soon as the first half lands
        for fh in range(2):
            nc.sync.dma_start(
                qk_all[:].rearrange("p (f c) -> p f c", f=NF)[:, fh * 4:(fh + 1) * 4, 0:512],
                qT[fh * 512:(fh + 1) * 512, 0:512].rearrange("(f p) c -> p f c", p=128))
        wk_all = const.tile([128, NF * HSL], BF16, tag="wk", name="wk_all")
        nc.sync.dma_start(
            wk_all[:], wk.rearrange("(f p) c -> p f c", p=128))
        bk_s = const.tile([128, 2], F32, tag="bk")
        nc.sync.dma_start(bk_s[:], bk[:])
        qk_wave(1)
        qk_wave(2)
        qk_wave(3)
        wv_all = wpool.tile([128, NF * VW], BF16, tag="wv", name="wv_all")
        nc.sync.dma_start(
            wv_all[:], wv.rearrange("(f p) c -> p f c", p=128))
        bva_s = const.tile([1, VW], BF16, tag="bva")
        nc.sync.dma_start(bva_s[:], bva[:])
        nc.gpsimd.partition_broadcast(bva_bc[:], bva_s[:])
        wq_s = [wq_all[:, f * HSL:(f + 1) * HSL] for f in range(NF)]
        wk_s = [wk_all[:, f * HSL:(f + 1) * HSL] for f in range(NF)]
        wv_s = [wv_all[:, f * VW:(f + 1) * VW] for f in range(NF)]

        # pair-packed projections: qP[p]/kP[p] [128, T], head 2p in rows
        # 0-63, head 2p+1 in rows 64-127 (enables S row-tiling, K=64 x2)
        qP = [const.tile([128, T], BF16, tag=f"qP{p}", name=f"qP{p}") for p in range(NP)]
        kP = [const.tile([128, T], BF16, tag=f"kP{p}", name=f"kP{p}") for p in range(NP)]
        v_sb = [const.tile([128, VW], BF16, tag=f"v{i}", name=f"v{i}") for i in range(NT)]
        # populated at round 8 of loop A, from the space wts frees
        oT, wo_s, late = [], [], {}

        def open_late_pools():
            lpool = ctx.enter_context(tc.tile_pool(name="late", bufs=1))
            late["rpool"] = ctx.enter_context(tc.tile_pool(name="recips", bufs=2))
            late["bpool"] = ctx.enter_context(tc.tile_pool(name="bcasts", bufs=2))
            late["ypool"] = ctx.enter_context(tc.tile_pool(name="yout", bufs=2))
            for g in range(2):
                oT.append(lpool.tile([128, T], BF16, tag=f"oT{g}", name=f"oT{g}"))
            wo_all = lpool.tile([128, 2 * E], BF16, tag="wo", name="wo_all")
            nc.sync.dma_start(
                wo_all[:], wo.rearrange("(g p) c -> p g c", p=128))
            wo_s.extend(wo_all[:, g * E:(g + 1) * E] for g in range(2))

        def proj_qk_group(w_tiles, bias_s, out_tiles, m, n):
            # rides the pss rotation (psav is reserved for AV accumulators)
            pst = pss.tile([128, 1024], F32, tag="s", name="projps")
            ps = pst[:, 0:512]
            for f in range(NF):
                nc.tensor.matmul(
                    ps, w_tiles[f][:, m * 128:(m + 1) * 128],
                    qk[f][:, n * 512:(n + 1) * 512],
                    start=(f == 0), stop=(f == NF - 1))
            nc.vector.tensor_scalar_add(
                out_tiles[m][:, n * 512:(n + 1) * 512], ps,
                bias_s[:, m:m + 1])

        def proj_v(it):
            # stationary = qT block, moving = wv; bias folded into the DVE
            # evacuation (tensor_add with the broadcast bias tile)
            ps = pss.tile([128, 1024], F32, tag="s", name="vps")
            pv = ps[:, 0:VW]
            for f in range(NF):
                nc.tensor.matmul(
                    pv, qk[f][:, it * 128:(it + 1) * 128], wv_s[f][:],
                    start=(f == 0), stop=(f == NF - 1))
            nc.vector.tensor_add(v_sb[it][:], pv, bva_bc[:])

        def s_exp(p, j, hf, pT_tiles):
            # S^T half-tiles for BOTH heads of pair p concurrently:
            # head 2p in PE rows 0-63, head 2p+1 in rows 64-127.
            psA = pss.tile([128, 1024], F32, tag="s", name=f"sA_{p}_{j}_{hf}")
            psB = pss.tile([128, 1024], F32, tag="s", name=f"sB_{p}_{j}_{hf}")
            for c in range(2):
                n = hf * 2 + c
                nc.tensor.matmul(
                    psA[:, c * 512:(c + 1) * 512],
                    kP[p][0:64, j * 128:(j + 1) * 128],
                    qP[p][0:64, n * 512:(n + 1) * 512],
                    start=True, stop=True)
            for c in range(2):
                n = hf * 2 + c
                nc.tensor.matmul(
                    psB[:, c * 512:(c + 1) * 512],
                    kP[p][64:128, j * 128:(j + 1) * 128],
                    qP[p][64:128, n * 512:(n + 1) * 512],
                    start=True, stop=True)
            ptA = ppool.tile([128, 1024], BF16, tag="pT", name=f"pT_{2*p}_{j}_{hf}")
            nc.scalar.activation(ptA[:], psA[:], ActFn.Exp, scale=0.125)
            pT_tiles[2 * p][j][hf] = ptA
            ptB = ppool.tile([128, 1024], BF16, tag="pT", name=f"pT_{2*p+1}_{j}_{hf}")
            nc.scalar.activation(ptB[:], psB[:], ActFn.Exp, scale=0.125)
            pT_tiles[2 * p + 1][j][hf] = ptB

        def av_unit(h, j, hf, av_set):
            # accumulate O.T chunks (i-half hf) for head h, v[j] stationary
            for c in range(2):
                nc.tensor.matmul(
                    av_set[c][0:HD + 1, :],
                    v_sb[j][:, h * 65:h * 65 + 65],
                    pT[h][j][hf][:, c * 512:(c + 1) * 512],
                    start=(j == 0), stop=(j == NT - 1))

        def av_normalize(h, hf, av_set):
            g, po = h // 2, (h % 2) * 64
            for c in range(2):
                n = 2 * hf + c
                recip = late["rpool"].tile([1, 512], F32, tag="recip", name="recip")
                nc.vector.reciprocal(recip[:], av_set[c][64:65, :])
                rbc = late["bpool"].tile([64, 512], F32, tag="rbc", name="rbc")
                nc.gpsimd.partition_broadcast(rbc[:], recip[:])
                nc.vector.tensor_mul(
                    oT[g][po:po + 64, n * 512:(n + 1) * 512],
                    av_set[c][0:64, :], rbc[:])

        # ---- lead: pair0 q-projection (all n) + k chunk 0 ----
        for n in range(NI):
            proj_qk_group(wq_s, bq_s, qP, 0, n)
        proj_qk_group(wk_s, bk_s, kP, 0, 0)

        pT = {h: [[None, None] for _ in range(NT)] for h in range(NHL)}

        # ---- loop A: S(P0) + v-proj + AV(h0/h1).half0 + proj fillers ----
        fillers = [(wk_s, bk_s, kP, 0, n) for n in range(1, NI)] + \
                  [(wq_s, bq_s, qP, 1, n) for n in range(NI)] + \
                  [(wk_s, bk_s, kP, 1, n) for n in range(NI)]
        av0 = [psav.tile([128, 512], F32, tag="ps", name=f"av_h0_f0_{c}")
               for c in range(2)]
        av1 = [psav.tile([128, 512], F32, tag="ps", name=f"av_h1_f0_{c}")
               for c in range(2)]
        for j in range(NT):
            s_exp(0, j, 0, pT)
            if j < 8:
                proj_v(2 * j)
                proj_v(2 * j + 1)
            s_exp(0, j, 1, pT)
            if j >= 2:
                av_unit(0, j - 2, 0, av0)
                av_unit(1, j - 2, 0, av1)
            # 2 filler groups/round so all 11 finish by round 5 (qk_all and
            # wv_all are freed at round 8; fillers read qk)
            for fi in (2 * j, 2 * j + 1):
                if fi < len(fillers):
                    proj_qk_group(*fillers[fi])
            if j == 8:
                wpool_cm.__exit__(None, None, None)
                open_late_pools()
        for j in range(NT - 2, NT):
            av_unit(0, j, 0, av0)
            av_unit(1, j, 0, av1)
        av_normalize(0, 0, av0)
        av_normalize(1, 0, av1)

        # ---- loop B: AV(h0/h1).half1 + S(P1) ----
        av0b = [psav.tile([128, 512], F32, tag="ps", name=f"av_h0_f1_{c}")
                for c in range(2)]
        av1b = [psav.tile([128, 512], F32, tag="ps", name=f"av_h1_f1_{c}")
                for c in range(2)]
        for j in range(NT):
            av_unit(0, j, 1, av0b)
            av_unit(1, j, 1, av1b)
            s_exp(1, j, 0, pT)
            s_exp(1, j, 1, pT)
        av_normalize(0, 1, av0b)
        av_normalize(1, 1, av1b)

        # ---- loop C: AV(h2/h3) both halves + out-projection ----
        av2 = pss.tile([128, 1024], F32, tag="s", name="av_h2_f0")
        av3 = pss.tile([128, 1024], F32, tag="s", name="av_h3_f0")
        av2b = [psav.tile([128, 512], F32, tag="ps", name=f"av_h2_f1_{c}")
                for c in range(2)]
        av3b = [psav.tile([128, 512], F32, tag="ps", name=f"av_h3_f1_{c}")
                for c in range(2)]
        av2_ = [av2[:, 0:512], av2[:, 512:1024]]
        av3_ = [av3[:, 0:512], av3[:, 512:1024]]
        for j in range(NT):
            av_unit(2, j, 0, av2_)
            av_unit(3, j, 0, av3_)
            av_unit(2, j, 1, av2b)
            av_unit(3, j, 1, av3b)
        av_normalize(2, 0, av2_)
        av_normalize(3, 0, av3_)
        av_normalize(2, 1, av2b)
        av_normalize(3, 1, av3b)

        # out-projection: both e-chunks accumulate in one 2-bank PSUM tile,
        # evacuated to f16 on DVE, then DMA'd out
        for it in range(NT):
            yt = late["ypool"].tile([128, E], mybir.dt.float16, tag="y", name="yt")
            ps = pss.tile([128, 1024], F32, tag="s", name="yps")
            for ec in range(2):
                for g in range(2):
                    nc.tensor.matmul(
                        ps[:, ec * 512:(ec + 1) * 512],
                        oT[g][:, it * 128:(it + 1) * 128],
                        wo_s[g][:, ec * 512:(ec + 1) * 512],
                        start=(g == 0), stop=(g == 1))
            nc.vector.tensor_copy(yt[:], ps[:])
            nc.sync.dma_start(y[it * 128:(it + 1) * 128, :], yt[:])


def build_nc(num_devices=8, reps=1):
    nc = bacc.Bacc("TRN2", target_bir_lowering=False, debug=False,
                   num_devices=num_devices)
    qT = nc.dram_tensor("qT", [E, T], BF16, kind="ExternalInput").ap()
    wq = nc.dram_tensor("wq", [E, HSL], BF16, kind="ExternalInput").ap()
    wk = nc.dram_tensor("wk", [E, HSL], BF16, kind="ExternalInput").ap()
    wv = nc.dram_tensor("wv", [E, VW], BF16, kind="ExternalInput").ap()
    wo = nc.dram_tensor("wo", [HSL, E], BF16, kind="ExternalInput").ap()
    bq = nc.dram_tensor("bq", [128, 2], F32, kind="ExternalInput").ap()
    bk = nc.dram_tensor("bk", [128, 2], F32, kind="ExternalInput").ap()
    bva = nc.dram_tensor("bva", [1, VW], BF16, kind="ExternalInput").ap()
    y = nc.dram_tensor("y", [T, E], mybir.dt.float16, kind="ExternalOutput").ap()
    with tile.TileContext(nc) as tc:
        for _ in range(reps):
            build_body(tc, qT, wq, wk, wv, wo, bq, bk, bva, y)
    nc.compile()
    return nc


# ---------------- host-side shard prep / gather ----------------

def eff_weight(mag, dirw, Am, Bm):
    Vu = dirw.astype(np.float32) + Bm.astype(np.float32) @ Am.astype(np.float32)
    c = np.float32(mag) / (np.linalg.norm(Vu) + np.float32(1e-8))
    return (c * Vu).astype(np.float32)


def make_in_maps(inputs):
    query = np.asarray(inputs["query"], np.float32)
    Wq = eff_weight(inputs["mag_q"], inputs["dir_q"], inputs["A_q"], inputs["B_q"])
    Wv = eff_weight(inputs["mag_v"], inputs["dir_v"], inputs["A_v"], inputs["B_v"])
    k_w = np.asarray(inputs["k_w"], np.float32)
    out_w = np.asarray(inputs["out_w"], np.float32)
    bias_q = np.asarray(inputs["bias_q"], np.float32)
    k_b = np.asarray(inputs["k_b"], np.float32)
    bias_v = np.asarray(inputs["bias_v"], np.float32)

    qT_b = [np.ascontiguousarray(query[:, b, :].T).astype(bf) for b in range(2)]
    WqT, WkT, WvT, WoT = Wq.T, k_w.T, Wv.T, out_w.T

    in_maps = []
    for c in range(8):
        b, h0 = c // 4, (c % 4) * 4
        cols = slice(h0 * HD, h0 * HD + HSL)
        wv_aug = np.zeros((E, VW), np.float32)
        bva = np.zeros((1, VW), np.float32)
        for hl in range(NHL):
            src = slice((h0 + hl) * HD, (h0 + hl + 1) * HD)
            dst = slice(hl * 65, hl * 65 + HD)
            wv_aug[:, dst] = WvT[:, src]
            bva[0, dst] = bias_v[src]
            bva[0, hl * 65 + HD] = 1.0
        in_maps.append({
            "qT": qT_b[b],
            "wq": np.ascontiguousarray(WqT[:, cols]).astype(bf),
            "wk": np.ascontiguousarray(WkT[:, cols]).astype(bf),
            "wv": wv_aug.astype(bf),
            "wo": np.ascontiguousarray(WoT[cols, :]).astype(bf),
            "bq": bias_q[cols].reshape(2, 128).T.copy(),
            "bk": k_b[cols].reshape(2, 128).T.copy(),
            "bva": bva.astype(bf),
        })
    return in_maps


def gather_output(results, inputs):
    # per-core partials are fp16 (halves the output-DMA tail); sum in fp32
    out_b = np.asarray(inputs["out_b"], np.float32)
    out = np.empty((T, 2, E), np.float32)
    for b in range(2):
        acc = results[4 * b]["y"].astype(np.float32)
        for c in range(4 * b + 1, 4 * b + 4):
            acc += results[c]["y"].astype(np.float32)
        out[:, b, :] = acc + out_b
    return out


# ---------------- public entry point ----------------

_CACHE = {}


class _Exec:
    def __init__(self, nc, n_cores=8):
        import jax
        from jax.sharding import Mesh, PartitionSpec
        from jax.experimental.shard_map import shard_map
        from concourse import mybir as _mb
        from concourse.bass2jax import (
            _bass_exec_p, install_neuronx_cc_hook, partition_id_tensor)

        install_neuronx_cc_hook()
        self.jax = jax
        self.n_cores = n_cores
        pname = nc.partition_id_tensor.name if nc.partition_id_tensor else None
        in_names, out_names, out_avals = [], [], []
        for alloc in nc.m.functions[0].allocations:
            if not isinstance(alloc, _mb.MemoryLocationSet):
                continue
            name = alloc.memorylocations[0].name
            if alloc.kind == "ExternalInput":
                if name != pname:
                    in_names.append(name)
            elif alloc.kind == "ExternalOutput":
                out_avals.append(jax.core.ShapedArray(
                    tuple(alloc.tensor_shape), _mb.dt.np(alloc.dtype)))
                out_names.append(name)
        self.in_names, self.out_names, self.out_avals = in_names, out_names, out_avals
        all_names = in_names + out_names + ([pname] if pname else [])

        def _body(*args):
            operands = list(args)
            if pname is not None:
                operands.append(partition_id_tensor())
            return tuple(_bass_exec_p.bind(
                *operands, out_avals=tuple(out_avals), in_names=tuple(all_names),
                out_names=tuple(out_names), lowering_input_output_aliases=(),
                sim_require_finite=True, sim_require_nnan=True, nc=nc))

        devices = jax.devices()[:n_cores]
        import numpy as _np
        self.mesh = Mesh(_np.asarray(devices), ("core",))
        nin = len(in_names) + len(out_names)
        self.fn = jax.jit(
            shard_map(_body, mesh=self.mesh, in_specs=(PartitionSpec("core"),) * nin,
                      out_specs=(PartitionSpec("core"),) * len(out_names),
                      check_rep=False),
            keep_unused=True)
        self.sharding = jax.sharding.NamedSharding(self.mesh, PartitionSpec("core"))

    def run(self, in_maps):
        jax = self.jax
        n = self.n_cores
        concat_in = [
            np.concatenate([np.asarray(in_maps[c][name]) for c in range(n)], axis=0)
            for name in self.in_names
        ]
        zeros = [np.zeros((n * a.shape[0], *a.shape[1:]), a.dtype)
                 for a in self.out_avals]
        args = [jax.device_put(x, self.sharding) for x in concat_in + zeros]
        outs = self.fn(*args)
        jax.block_until_ready(outs)
        return [
            {name: np.asarray(outs[i]).reshape(n, *self.out_avals[i].shape)[c]
             for i, name in enumerate(self.out_names)}
            for c in range(n)
        ]


def _get_exec():
    if "exec" not in _CACHE:
        _CACHE["exec"] = _Exec(build_nc(num_devices=8, reps=1))
    return _CACHE["exec"]


def kernel(**inputs):
    """Full-input, full-output DoRA multihead attention on 8 NeuronCores.

    Shards 32 (batch, head) units across 8 cores (4 heads each); host
    reconstructs the (tiny) DoRA effective weights, pre-transposes the
    per-batch query to bf16, and sums the 4 per-core output partials per
    batch (+ out_b) at the end.
    """
    import time as _time

    inputs = {k: np.asarray(v) for k, v in inputs.items()}
    in_maps = make_in_maps(inputs)
    last_err = None
    for _attempt in range(6):
        try:
            ex = _get_exec()
            results = ex.run(in_maps)
            break
        except Exception as e:  # transient device errors observed on axon
            last_err = e
            _CACHE.pop("exec", None)
            _time.sleep(4.0 * (_attempt + 1))
    else:
        raise last_err
    return gather_output(results, inputs)


# revision 11
# speedup vs baseline: 2.6038x; 2.6038x over previous
"""DoRA multihead attention TRN2 kernel, v3: row-tiled S head pairs.

Per core (4 heads = 2 pairs, one batch). Heads of a pair live in partition
halves (rows 0-63 / 64-127) of shared q/k tiles, so the two K=64 S matmuls
run CONCURRENTLY in disjoint PE row-groups (tile_position via base_partition)
-- S costs half the streaming cycles of the padded-K=128 form.

Schedule (PSUM banks: S rotation 4 + AV half-sets 4 = 8):
  lead:   DMA loads, q-proj pair0 (n0..3), k-proj pair0 (n0)
  A(j):   S(P0,j) 2 halves; v-proj (2/round, rounds 0-7); AV(h0,j-2).half0;
          AV(h1,j-2).half0; proj filler (k P0 n1-3, q/k P1)
  B(j):   AV(h0,j).half1; AV(h1,j).half1; S(P1,j)
  C:      AV(h2/h3) both halves (8 banks), out-projection, DMA out
ACT does exp only (the global bottleneck: ~133us); all other PSUM
evacuation is on DVE/gpsimd.
"""
import sys
if "/opt/trn_rl_repo" not in sys.path:
    sys.path.insert(0, "/opt/trn_rl_repo")

import numpy as np
import ml_dtypes
from contextlib import ExitStack

import concourse.bass as bass
import concourse.tile as tile
from concourse import bacc, mybir

bf = ml_dtypes.bfloat16
BF16, F32 = mybir.dt.bfloat16, mybir.dt.float32
ActFn = mybir.ActivationFunctionType

T = 2048
E = 1024
HD = 64
NHL = 4                 # heads per core
NP = 2                  # head pairs per core
HSL = NHL * HD          # 256
VW = NHL * (HD + 1)     # 260
NT = T // 128           # 16
NF = E // 128           # 8
NI = T // 512           # 4


def build_body(tc, qT, wq, wk, wv, wo, bq, bk, bva, y):
    nc = tc.nc
    with ExitStack() as ctx:
        const = ctx.enter_context(tc.tile_pool(name="const", bufs=1))
        pss = ctx.enter_context(tc.tile_pool(name="pss", bufs=2, space="PSUM"))
        psw = ctx.enter_context(tc.tile_pool(name="psw", bufs=4, space="PSUM"))
        # pT pools split by i-half so pool-FIFO reuse in loop B lands on
        # tiles whose AV readers were already emitted (half0 read in B
        # lockstep, half1 read during loop A rounds 8-15)
        ppH0 = ctx.enter_context(tc.tile_pool(name="pTh0", bufs=32))
        ppH1 = ctx.enter_context(tc.tile_pool(name="pTh1", bufs=32))
        wpool_cm = tc.tile_pool(name="wts", bufs=1)
        wpool = wpool_cm.__enter__()

        # ---- input loads ----
        bva_bc = const.tile([128, VW], BF16, tag="bva_bc")
        wq_all = const.tile([128, NF * HSL], BF16, tag="wq", name="wq_all")
        nc.sync.dma_start(
            wq_all[:], wq.rearrange("(f p) c -> p f c", p=128))
        bq_s = const.tile([128, 2], F32, tag="bq")
        nc.sync.dma_start(bq_s[:], bq[:])
        qk_all = wpool.tile([128, NF * T], BF16, tag="qTs", name="qk_all")
        qk = [qk_all[:, f * T:(f + 1) * T] for f in range(NF)]

        def qk_wave(n):
            nc.sync.dma_start(
                qk_all[:].rearrange("p (f c) -> p f c", f=NF)[:, :, n * 512:(n + 1) * 512],
                qT[:, n * 512:(n + 1) * 512].rearrange("(f p) c -> p f c", p=128))

        for fh in range(2):
            nc.sync.dma_start(
                qk_all[:].rearrange("p (f c) -> p f c", f=NF)[:, fh * 4:(fh + 1) * 4, 0:512],
                qT[fh * 512:(fh + 1) * 512, 0:512].rearrange("(f p) c -> p f c", p=128))
        wk_all = const.tile([128, NF * HSL], BF16, tag="wk", name="wk_all")
        nc.sync.dma_start(
            wk_all[:], wk.rearrange("(f p) c -> p f c", p=128))
        bk_s = const.tile([128, 2], F32, tag="bk")
        nc.sync.dma_start(bk_s[:], bk[:])
        qk_wave(1)
        qk_wave(2)
        qk_wave(3)
        wv_all = wpool.tile([128, NF * VW], BF16, tag="wv", name="wv_all")
        nc.sync.dma_start(
            wv_all[:], wv.rearrange("(f p) c -> p f c", p=128))
        bva_s = const.tile([1, VW], BF16, tag="bva")
        nc.sync.dma_start(bva_s[:], bva[:])
        nc.gpsimd.partition_broadcast(bva_bc[:], bva_s[:])
        wq_s = [wq_all[:, f * HSL:(f + 1) * HSL] for f in range(NF)]
        wk_s = [wk_all[:, f * HSL:(f + 1) * HSL] for f in range(NF)]
        wv_s = [wv_all[:, f * VW:(f + 1) * VW] for f in range(NF)]

        # pair-packed projections: head 2p in rows 0-63, head 2p+1 in rows
        # 64-127 (lets the two K=64 S matmuls run in disjoint PE row-groups)
        qP = [const.tile([128, T], BF16, tag=f"qP{p}", name=f"qP{p}") for p in range(NP)]
        kP = [const.tile([128, T], BF16, tag=f"kP{p}", name=f"kP{p}") for p in range(NP)]
        v_sb = [const.tile([128, VW], BF16, tag=f"v{i}", name=f"v{i}") for i in range(NT)]
        oT, wo_s, late = [], [], {}

        def open_late_pools():
            lpool = ctx.enter_context(tc.tile_pool(name="late", bufs=1))
            late["rpool"] = ctx.enter_context(tc.tile_pool(name="recips", bufs=2))
            late["bpool"] = ctx.enter_context(tc.tile_pool(name="bcasts", bufs=2))
            late["ypool"] = ctx.enter_context(tc.tile_pool(name="yout", bufs=2))
            for g in range(2):
                oT.append(lpool.tile([128, T], BF16, tag=f"oT{g}", name=f"oT{g}"))
            wo_all = lpool.tile([128, 2 * E], BF16, tag="wo", name="wo_all")
            nc.sync.dma_start(
                wo_all[:], wo.rearrange("(g p) c -> p g c", p=128))
            wo_s.extend(wo_all[:, g * E:(g + 1) * E] for g in range(2))

        def proj_qk_group(w_tiles, bias_s, out_tiles, m, n):
            ps = psw.tile([128, 512], F32, tag="ps", name="projps")
            for f in range(NF):
                nc.tensor.matmul(
                    ps[:], w_tiles[f][:, m * 128:(m + 1) * 128],
                    qk[f][:, n * 512:(n + 1) * 512],
                    start=(f == 0), stop=(f == NF - 1))
            nc.vector.tensor_scalar_add(
                out_tiles[m][:, n * 512:(n + 1) * 512], ps[:],
                bias_s[:, m:m + 1])

        def proj_v(it):
            ps = psw.tile([128, 512], F32, tag="ps", name="vps")
            pv = ps[:, 0:VW]
            for f in range(NF):
                nc.tensor.matmul(
                    pv, qk[f][:, it * 128:(it + 1) * 128], wv_s[f][:],
                    start=(f == 0), stop=(f == NF - 1))
            nc.vector.tensor_add(v_sb[it][:], pv, bva_bc[:])

        def s_exp(p, j, hf, pT_tiles):
            # S^T half-tiles for BOTH heads of pair p concurrently via
            # PE row-tiling (K=64 each, disjoint row groups)
            psA = pss.tile([128, 1024], F32, tag="s", name=f"sA_{p}_{j}_{hf}")
            psB = pss.tile([128, 1024], F32, tag="s", name=f"sB_{p}_{j}_{hf}")
            for c in range(2):
                n = hf * 2 + c
                nc.tensor.matmul(
                    psA[:, c * 512:(c + 1) * 512],
                    kP[p][0:64, j * 128:(j + 1) * 128],
                    qP[p][0:64, n * 512:(n + 1) * 512],
                    start=True, stop=True)
            for c in range(2):
                n = hf * 2 + c
                nc.tensor.matmul(
                    psB[:, c * 512:(c + 1) * 512],
                    kP[p][64:128, j * 128:(j + 1) * 128],
                    qP[p][64:128, n * 512:(n + 1) * 512],
                    start=True, stop=True)
            pool = ppH0 if hf == 0 else ppH1
            ptA = pool.tile([128, 1024], BF16, tag="pT", name=f"pT_{2*p}_{j}_{hf}")
            nc.scalar.activation(ptA[:], psA[:], ActFn.Exp, scale=0.125)
            pT_tiles[2 * p][j][hf] = ptA
            ptB = pool.tile([128, 1024], BF16, tag="pT", name=f"pT_{2*p+1}_{j}_{hf}")
            nc.scalar.activation(ptB[:], psB[:], ActFn.Exp, scale=0.125)
            pT_tiles[2 * p + 1][j][hf] = ptB

        def av_unit(h, j, hf, av_set):
            for c in range(2):
                nc.tensor.matmul(
                    av_set[c][0:HD + 1, :],
                    v_sb[j][:, h * 65:h * 65 + 65],
                    pT[h][j][hf][:, c * 512:(c + 1) * 512],
                    start=(j == 0), stop=(j == NT - 1))

        def av_normalize(h, hf, av_set):
            g, po = h // 2, (h % 2) * 64
            for c in range(2):
                n = 2 * hf + c
                recip = late["rpool"].tile([1, 512], F32, tag="recip", name="recip")
                nc.vector.reciprocal(recip[:], av_set[c][64:65, :])
                rbc = late["bpool"].tile([64, 512], F32, tag="rbc", name="rbc")
                nc.gpsimd.partition_broadcast(rbc[:], recip[:])
                nc.vector.tensor_mul(
                    oT[g][po:po + 64, n * 512:(n + 1) * 512],
                    av_set[c][0:64, :], rbc[:])

        # ---- lead: pair0 q-projection (all n) + k chunk 0 ----
        for n in range(NI):
            proj_qk_group(wq_s, bq_s, qP, 0, n)
        proj_qk_group(wk_s, bk_s, kP, 0, 0)

        pT = {h: [[None, None] for _ in range(NT)] for h in range(NHL)}

        # ---- loop A: S(P0) + v-proj + fillers + AV(h0/h1).half1 ----
        fillers = [(wk_s, bk_s, kP, 0, n) for n in range(1, NI)] + \
                  [(wq_s, bq_s, qP, 1, n) for n in range(NI)] + \
                  [(wk_s, bk_s, kP, 1, n) for n in range(NI)]
        av0b = av1b = None
        avq = [(h, j) for j in range(NT) for h in (0, 1)]  # half1 queue
        for j in range(NT):
            s_exp(0, j, 0, pT)
            if j < 8:
                proj_v(2 * j)
            s_exp(0, j, 1, pT)
            if j < 8:
                proj_v(2 * j + 1)
            for fi in (2 * j, 2 * j + 1):
                if fi < len(fillers):
                    proj_qk_group(*fillers[fi])
            if j == 7:
                wpool_cm.__exit__(None, None, None)
                open_late_pools()
                av0b = [psw.tile([128, 512], F32, tag="ps", name=f"av_h0_f1_{c}")
                        for c in range(2)]
                av1b = [psw.tile([128, 512], F32, tag="ps", name=f"av_h1_f1_{c}")
                        for c in range(2)]
            if j >= 8:
                for _ in range(4):
                    if avq:
                        h, jj = avq.pop(0)
                        av_unit(h, jj, 1, av0b if h == 0 else av1b)
        while avq:
            h, jj = avq.pop(0)
            av_unit(h, jj, 1, av0b if h == 0 else av1b)
        av_normalize(0, 1, av0b)
        av_normalize(1, 1, av1b)

        # ---- loop B: AV(h0/h1).half0 lockstep + S(P1) ----
        av0 = [psw.tile([128, 512], F32, tag="ps", name=f"av_h0_f0_{c}")
               for c in range(2)]
        av1 = [psw.tile([128, 512], F32, tag="ps", name=f"av_h1_f0_{c}")
               for c in range(2)]
        for r in range(NT):
            av_unit(0, r, 0, av0)
            av_unit(1, r, 0, av1)
            s_exp(1, r, 0, pT)
            s_exp(1, r, 1, pT)
        av_normalize(0, 0, av0)
        av_normalize(1, 0, av1)

        # ---- loop C: AV(h2/h3) + out-projection ----
        av2 = [psw.tile([128, 512], F32, tag="ps", name=f"av_h2_f0_{c}")
               for c in range(2)]
        av3 = [psw.tile([128, 512], F32, tag="ps", name=f"av_h3_f0_{c}")
               for c in range(2)]
        for j in range(NT):
            av_unit(2, j, 0, av2)
            av_unit(3, j, 0, av3)
        av_normalize(2, 0, av2)
        av_normalize(3, 0, av3)
        av2b = [psw.tile([128, 512], F32, tag="ps", name=f"av_h2_f1_{c}")
                for c in range(2)]
        av3b = [psw.tile([128, 512], F32, tag="ps", name=f"av_h3_f1_{c}")
                for c in range(2)]

        def outproj(it):
            yt = late["ypool"].tile([128, E], mybir.dt.float16, tag="y", name="yt")
            ps = pss.tile([128, 1024], F32, tag="s", name="yps")
            for ec in range(2):
                for g in range(2):
                    nc.tensor.matmul(
                        ps[:, ec * 512:(ec + 1) * 512],
                        oT[g][:, it * 128:(it + 1) * 128],
                        wo_s[g][:, ec * 512:(ec + 1) * 512],
                        start=(g == 0), stop=(g == 1))
            nc.vector.tensor_copy(yt[:], ps[:])
            nc.sync.dma_start(y[it * 128:(it + 1) * 128, :], yt[:])

        # half1 AV interleaved with the first 8 out-proj tiles (they only
        # need i < 1024 = the just-normalized half0 columns of oT)
        for j in range(NT):
            av_unit(2, j, 1, av2b)
            av_unit(3, j, 1, av3b)
            if j % 2 == 1:
                outproj(j // 2)
        av_normalize(2, 1, av2b)
        av_normalize(3, 1, av3b)
        for it in range(8, NT):
            outproj(it)


def build_nc(num_devices=8, reps=1):
    nc = bacc.Bacc("TRN2", target_bir_lowering=False, debug=False,
                   num_devices=num_devices)
    qT = nc.dram_tensor("qT", [E, T], BF16, kind="ExternalInput").ap()
    wq = nc.dram_tensor("wq", [E, HSL], BF16, kind="ExternalInput").ap()
    wk = nc.dram_tensor("wk", [E, HSL], BF16, kind="ExternalInput").ap()
    wv = nc.dram_tensor("wv", [E, VW], BF16, kind="ExternalInput").ap()
    wo = nc.dram_tensor("wo", [HSL, E], BF16, kind="ExternalInput").ap()
    bq = nc.dram_tensor("bq", [128, 2], F32, kind="ExternalInput").ap()
    bk = nc.dram_tensor("bk", [128, 2], F32, kind="ExternalInput").ap()
    bva = nc.dram_tensor("bva", [1, VW], BF16, kind="ExternalInput").ap()
    y = nc.dram_tensor("y", [T, E], mybir.dt.float16, kind="ExternalOutput").ap()
    with tile.TileContext(nc) as tc:
        for _ in range(reps):
            build_body(tc, qT, wq, wk, wv, wo, bq, bk, bva, y)
    nc.compile()
    return nc


# ---------------- host-side shard prep / gather ----------------

def eff_weight(mag, dirw, Am, Bm):
    Vu = dirw.astype(np.float32) + Bm.astype(np.float32) @ Am.astype(np.float32)
    c = np.float32(mag) / (np.linalg.norm(Vu) + np.float32(1e-8))
    return (c * Vu).astype(np.float32)


def make_in_maps(inputs):
    query = np.asarray(inputs["query"], np.float32)
    Wq = eff_weight(inputs["mag_q"], inputs["dir_q"], inputs["A_q"], inputs["B_q"])
    Wv = eff_weight(inputs["mag_v"], inputs["dir_v"], inputs["A_v"], inputs["B_v"])
    k_w = np.asarray(inputs["k_w"], np.float32)
    out_w = np.asarray(inputs["out_w"], np.float32)
    bias_q = np.asarray(inputs["bias_q"], np.float32)
    k_b = np.asarray(inputs["k_b"], np.float32)
    bias_v = np.asarray(inputs["bias_v"], np.float32)

    qT_b = [np.ascontiguousarray(query[:, b, :].T).astype(bf) for b in range(2)]
    WqT, WkT, WvT, WoT = Wq.T, k_w.T, Wv.T, out_w.T

    in_maps = []
    for c in range(8):
        b, h0 = c // 4, (c % 4) * 4
        cols = slice(h0 * HD, h0 * HD + HSL)
        wv_aug = np.zeros((E, VW), np.float32)
        bva = np.zeros((1, VW), np.float32)
        for hl in range(NHL):
            src = slice((h0 + hl) * HD, (h0 + hl + 1) * HD)
            dst = slice(hl * 65, hl * 65 + HD)
            wv_aug[:, dst] = WvT[:, src]
            bva[0, dst] = bias_v[src]
            bva[0, hl * 65 + HD] = 1.0
        in_maps.append({
            "qT": qT_b[b],
            "wq": np.ascontiguousarray(WqT[:, cols]).astype(bf),
            "wk": np.ascontiguousarray(WkT[:, cols]).astype(bf),
            "wv": wv_aug.astype(bf),
            "wo": np.ascontiguousarray(WoT[cols, :]).astype(bf),
            "bq": bias_q[cols].reshape(2, 128).T.copy(),
            "bk": k_b[cols].reshape(2, 128).T.copy(),
            "bva": bva.astype(bf),
        })
    return in_maps


def gather_output(results, inputs):
    # per-core partials are fp16 (halves the output-DMA tail); sum in fp32
    out_b = np.asarray(inputs["out_b"], np.float32)
    out = np.empty((T, 2, E), np.float32)
    for b in range(2):
        acc = results[4 * b]["y"].astype(np.float32)
        for c in range(4 * b + 1, 4 * b + 4):
            acc += results[c]["y"].astype(np.float32)
        out[:, b, :] = acc + out_b
    return out


# ---------------- public entry point ----------------

_CACHE = {}


class _Exec:
    def __init__(self, nc, n_cores=8):
        import jax
        from jax.sharding import Mesh, PartitionSpec
        from jax.experimental.shard_map import shard_map
        from concourse import mybir as _mb
        from concourse.bass2jax import (
            _bass_exec_p, install_neuronx_cc_hook, partition_id_tensor)

        install_neuronx_cc_hook()
        self.jax = jax
        self.n_cores = n_cores
        pname = nc.partition_id_tensor.name if nc.partition_id_tensor else None
        in_names, out_names, out_avals = [], [], []
        for alloc in nc.m.functions[0].allocations:
            if not isinstance(alloc, _mb.MemoryLocationSet):
                continue
            name = alloc.memorylocations[0].name
            if alloc.kind == "ExternalInput":
                if name != pname:
                    in_names.append(name)
            elif alloc.kind == "ExternalOutput":
                out_avals.append(jax.core.ShapedArray(
                    tuple(alloc.tensor_shape), _mb.dt.np(alloc.dtype)))
                out_names.append(name)
        self.in_names, self.out_names, self.out_avals = in_names, out_names, out_avals
        all_names = in_names + out_names + ([pname] if pname else [])

        def _body(*args):
            operands = list(args)
            if pname is not None:
                operands.append(partition_id_tensor())
            return tuple(_bass_exec_p.bind(
                *operands, out_avals=tuple(out_avals), in_names=tuple(all_names),
                out_names=tuple(out_names), lowering_input_output_aliases=(),
                sim_require_finite=True, sim_require_nnan=True, nc=nc))

        devices = jax.devices()[:n_cores]
        import numpy as _np
        self.mesh = Mesh(_np.asarray(devices), ("core",))
        nin = len(in_names) + len(out_names)
        self.fn = jax.jit(
            shard_map(_body, mesh=self.mesh, in_specs=(PartitionSpec("core"),) * nin,
                      out_specs=(PartitionSpec("core"),) * len(out_names),
                      check_rep=False),
            keep_unused=True)
        self.sharding = jax.sharding.NamedSharding(self.mesh, PartitionSpec("core"))

    def run(self, in_maps):
        jax = self.jax
        n = self.n_cores
        concat_in = [
            np.concatenate([np.asarray(in_maps[c][name]) for c in range(n)], axis=0)
            for name in self.in_names
        ]
        zeros = [np.zeros((n * a.shape[0], *a.shape[1:]), a.dtype)
                 for a in self.out_avals]
        args = [jax.device_put(x, self.sharding) for x in concat_in + zeros]
        outs = self.fn(*args)
        jax.block_until_ready(outs)
        return [
            {name: np.asarray(outs[i]).reshape(n, *self.out_avals[i].shape)[c]
             for i, name in enumerate(self.out_names)}
            for c in range(n)
        ]


def _get_exec():
    if "exec" not in _CACHE:
        _CACHE["exec"] = _Exec(build_nc(num_devices=8, reps=1))
    return _CACHE["exec"]


def kernel(**inputs):
    """Full-input, full-output DoRA multihead attention on 8 NeuronCores.

    Shards 32 (batch, head) units across 8 cores (4 heads each); host
    reconstructs the (tiny) DoRA effective weights, pre-transposes the
    per-batch query to bf16, and sums the 4 per-core output partials per
    batch (+ out_b) at the end.
    """
    import time as _time

    inputs = {k: np.asarray(v) for k, v in inputs.items()}
    in_maps = make_in_maps(inputs)
    last_err = None
    for _attempt in range(6):
        try:
            ex = _get_exec()
            results = ex.run(in_maps)
            break
        except Exception as e:  # transient device errors observed on axon
            last_err = e
            _CACHE.pop("exec", None)
            _time.sleep(4.0 * (_attempt + 1))
    else:
        raise last_err
    return gather_output(results, inputs)
